# revision 50
# baseline (speedup 1.0000x reference)
"""AttentiveDensenet Trainium2 Bass kernel (v2).

Data-parallel over batch B=8 across 8 NeuronCores (1 image per core).

v2 changes vs v1 (driven by trace analysis of the 1.52ms baseline):
  - Conv weights are host-packed so each layer's w1/w2 load as ONE big
    DMA each into resident SBUF tiles (double-buffered, prefetched a
    layer ahead). Kills the per-tile weight DMA storm (1392 SP DMA
    issues, 299k tiny packets) that starved the PE and kept the HAM
    clock gate cold (PE ran at 1.2 GHz for 87% of the run).
  - o is transposed to channel-major with PE transpose-mode matmuls
    (32x [128,128] blocks/layer) + ACT evac into the padded conv input,
    replacing the DRAM bounce + xbar-transpose chain (~40us/layer of
    dead time).
  - Attention is emitted per quarter-image (2 pos-blocks) and conv1 in
    8-row chunks interleaved with the o-transposes, so DVE attention
    overlaps PE conv work instead of serializing.
  - Weighted sum accumulates bf16 products with a pair tree (2x DVE
    mode) instead of sequential f32 adds.
  - A dummy AllGather at kernel start absorbs the ~47us core-launch
    skew barrier under layer-0 compute; per-layer BN stat AllGathers
    then run near their intrinsic latency.
  - BN stats are reduced per conv1-chunk as results land (DVE is idle
    then), so the AllGather starts immediately after the last chunk.
"""
import numpy as np
import ml_dtypes

import concourse.bacc as bacc
import concourse.mybir as mybir
import concourse.tile as tile
from concourse import bass_utils

L, C, B, H, W = 4, 256, 8, 32, 32
NH, KD = 8, 64
KH = NH * KD          # 512
HW = H * W            # 1024
P = 128
NC = 8                # cores
TOPK = 4
EPS = 1e-7
BN_EPS = 1e-5
PW = W + 2            # 34
PHW = PW * (H + 2)    # 1156
NQ = 4                # quarters (2 pos-blocks each)
RPC = 8               # conv chunk rows
NCH = H // RPC        # 4 conv chunks
CW = PW * RPC         # 272 conv chunk width (incl pad cols)

f32 = mybir.dt.float32
bf16 = mybir.dt.bfloat16
AX = mybir.AxisListType
OP = mybir.AluOpType
ACTF = mybir.ActivationFunctionType

_compiled = {}


def _build(ncores=NC, layers=L, stages=99):
    nc = bacc.Bacc(None, target_bir_lowering=False, debug=False, num_devices=ncores)

    # ---- DRAM I/O (per-core shapes; weights replicated) ----
    xin = nc.dram_tensor("xin", [C, HW], f32, kind="ExternalInput").ap()
    wq = nc.dram_tensor("wq", [L, P, 2 * KH], bf16, kind="ExternalInput").ap()
    wk = nc.dram_tensor("wk", [L, P, 2 * KH], bf16, kind="ExternalInput").ap()
    wv = nc.dram_tensor("wv", [L, P, 2 * KH], bf16, kind="ExternalInput").ap()
    bq = nc.dram_tensor("bq", [L, 1, KH], bf16, kind="ExternalInput").ap()
    bk = nc.dram_tensor("bk", [L, 1, KH], bf16, kind="ExternalInput").ap()
    bv = nc.dram_tensor("bv", [L, 1, KH], bf16, kind="ExternalInput").ap()
    w1d = nc.dram_tensor("w1d", [L, P, 72 * P], bf16, kind="ExternalInput").ap()
    w2d = nc.dram_tensor("w2d", [L, P, 36 * P], bf16, kind="ExternalInput").ap()
    bngd = nc.dram_tensor("bngd", [L, P, 2], f32, kind="ExternalInput").ap()
    bnbd = nc.dram_tensor("bnbd", [L, P, 2], f32, kind="ExternalInput").ap()
    gob2d = nc.dram_tensor("gob2d", [L, P, 2], f32, kind="ExternalInput").ap()
    gamd = nc.dram_tensor("gamd", [L, P, 1], f32, kind="ExternalInput").ap()
    identd = nc.dram_tensor("identd", [P, P], bf16, kind="ExternalInput").ap()
    out = nc.dram_tensor("out", [C, HW], f32, kind="ExternalOutput").ap()

    with tile.TileContext(nc) as tc:
        with tc.tile_pool(name="main", bufs=1) as mp, \
             tc.tile_pool(name="prodp", bufs=6) as prodp, \
             tc.tile_pool(name="sprod", bufs=2) as sprod, \
             tc.tile_pool(name="wkvp", bufs=4) as wkvp, \
             tc.tile_pool(name="biasp", bufs=3) as biasp, \
             tc.tile_pool(name="kqvps", bufs=2, space="PSUM") as kqvps, \
             tc.tile_pool(name="convps", bufs=4, space="PSUM") as convps, \
             tc.tile_pool(name="xps", bufs=2, space="PSUM") as xps, \
             tc.tile_pool(name="dramp", bufs=2, space="DRAM") as dramp:

            # ---- persistent tiles ----
            x = [mp.tile([P, HW], f32, name=f"x{i}") for i in range(2)]
            xb = [mp.tile([P, HW], bf16, name=f"xb{i}") for i in range(2)]
            qbt = mp.tile([P, 8 * KH], bf16, name="qbt")
            kbt = [mp.tile([P, 8 * KH], bf16, name=f"kbt{i}") for i in range(L)]
            vbt = [mp.tile([P, 8 * KH], bf16, name=f"vbt{i}") for i in range(L)]
            S = mp.tile([P, 64 * 5], f32, name="S")
            attn = mp.tile([P, 64 * 5], f32, name="attn")
            attnb = mp.tile([P, 64 * 5], bf16, name="attnb")
            mx = mp.tile([P, 64], f32, name="mx")
            mx2 = mp.tile([P, 64], f32, name="mx2")
            zs = mp.tile([P, 64], f32, name="zs")
            dmin = mp.tile([P, 64], f32, name="dmin")
            mxp = mp.tile([P, 64], f32, name="mxp")
            o = mp.tile([P, 8 * KH], bf16, name="o")
            opad = [mp.tile([P, PHW + 2], bf16, name=f"opad{i}") for i in range(4)]
            y1 = [mp.tile([P, HW], f32, name=f"y1_{i}") for i in range(2)]
            h1p = [mp.tile([P, PHW + 2], bf16, name=f"h1p{i}") for i in range(2)]
            st = mp.tile([P, 16], f32, name="st")       # (co, kind, chunk)
            st2 = mp.tile([P, 4], f32, name="st2")      # (co, kind)
            stT = mp.tile([4, P], f32, name="stT")      # transposed stats
            gst8 = mp.tile([NC, 512], f32, name="gst8")  # gathered rows
            gsum = mp.tile([P, 4], f32, name="gsum")
            ones1 = mp.tile([1, P], bf16, name="ones1")
            ident = mp.tile([P, P], bf16, name="ident")
            identf = mp.tile([P, P], f32, name="identf")
            wres = mp.tile([1, 8], f32, name="wres")
            wsrc = mp.tile([1, 8], f32, name="wsrc")
            # resident conv weights, double-buffered across layers
            w1t = [mp.tile([P, 72 * P], bf16, name=f"w1t{i}") for i in range(2)]
            w2t = [mp.tile([P, 36 * P], bf16, name=f"w2t{i}") for i in range(2)]
            # per-layer consts (columns = co half)
            bngt = mp.tile([P, 2], f32, name="bngt")
            bnbt = mp.tile([P, 2], f32, name="bnbt")
            gob2t = mp.tile([P, 2], f32, name="gob2t")
            gamt = mp.tile([P, 1], f32, name="gamt")
            # BN scratch (columns = co half)
            mean2 = mp.tile([P, 2], f32, name="mean2")
            vart = mp.tile([P, 2], f32, name="vart")
            sq2 = mp.tile([P, 2], f32, name="sq2")
            stdt = mp.tile([P, 2], f32, name="stdt")
            A2 = mp.tile([P, 2], f32, name="A2")
            B2 = mp.tile([P, 2], f32, name="B2")

            # ---- init ----
            for i in range(2):
                nc.sync.dma_start(x[i][:], xin[i * P:(i + 1) * P, :])
                nc.scalar.copy(xb[i][:], x[i][:])
            nc.sync.dma_start(ident[:], identd)
            nc.vector.tensor_copy(identf[:], ident[:])
            for i in range(4):
                nc.vector.memset(opad[i][:], 0)
            for i in range(2):
                nc.vector.memset(h1p[i][:], 0)
            nc.vector.memset(ones1[:], 1.0)
            nc.vector.memset(S[:], 0)
            nc.vector.memset(attn[:], 0)
            nc.vector.memset(attnb[:], 0)
            nc.vector.memset(wsrc[:], 0)

            # PE warmup: ~5us of dummy matmuls (zeros) so the HAM clock
            # gate reaches 2.4 GHz before the first real KQV matmul.
            # Their (zero) result is consumed additively at the end.
            wscr = mp.tile([1, 512], bf16, name="wscr")
            for wi in range(16):
                wps = kqvps.tile([P, KH], f32, name="kqv_ps")
                nc.tensor.matmul(wps[:, 0:320], ones1[:], attnb[0:1, 0:320],
                                 start=True, stop=False)
                nc.tensor.matmul(wps[:, 0:320], ones1[:], attnb[0:1, 0:320],
                                 start=False, stop=True)
                if wi == 15:
                    nc.scalar.copy(wscr[0:1, 0:320], wps[0:1, 0:320])

            # warmup collective: absorbs core-launch skew + CC init
            # barrier while layer-0 compute runs. wres (all zeros) is
            # consumed additively just before the output DMA so DCE
            # can't drop the chain.
            wrmi = dramp.tile([1, 8], f32, name="wrmi")
            wrmo = dramp.tile([ncores, 8], f32, name="wrmo", addr_space="Shared")
            nc.sync.dma_start(wrmi[:], wsrc[:])
            nc.gpsimd.collective_compute(
                "AllGather", OP.bypass,
                replica_groups=[list(range(ncores))],
                ins=[wrmi.opt()], outs=[wrmo.opt()])
            nc.sync.dma_start(wres[:], wrmo[0:1, :])



            S3 = S[:].rearrange("p (g t) -> p g t", t=5)
            at3 = attn[:].rearrange("p (g t) -> p g t", t=5)
            ab3 = attnb[:].rearrange("p (g t) -> p g t", t=5)


            for l in range(layers):
                R = l + 1      # number of real keys
                T = R + 1      # +1 zero key

                # ---- per-layer consts ----
                nc.sync.dma_start(bngt[:], bngd[l])
                nc.sync.dma_start(bnbt[:], bnbd[l])
                nc.sync.dma_start(gob2t[:], gob2d[l])
                nc.sync.dma_start(gamt[:], gamd[l])

                # ---- K/Q/V 1x1 convs, position-major ----
                wts, bts = {}, {}
                for name, wdr, bdr in (("k", wk, bk), ("v", wv, bv), ("q", wq, bq)):
                    bt = biasp.tile([1, KH], bf16, name=f"bias_{name}", tag="bias")
                    nc.sync.dma_start(bt[:], bdr[l])
                    wt = wkvp.tile([P, 2 * KH], bf16, name=f"w_{name}", tag="wkv")
                    nc.sync.dma_start(wt[:], wdr[l])
                    wts[name], bts[name] = wt, bt
                if l == 0:
                    # layer-0 conv weights (after the KQV weight DMAs so
                    # those win the queue race; conv1 needs these ~40us in)
                    nc.sync.dma_start(w1t[0][:], w1d[0])
                    nc.sync.dma_start(w2t[0][:], w2d[0])
                dests = {"k": kbt[l][:], "v": vbt[l][:], "q": qbt[:]}
                for pb in range(8):
                    for name in ("k", "v", "q"):
                        ps = kqvps.tile([P, KH], f32, name="kqv_ps")
                        nc.tensor.matmul(ps[:], ones1[:], bts[name][:],
                                         start=True, stop=False)
                        nc.tensor.matmul(ps[:], xb[0][:, pb * P:(pb + 1) * P],
                                         wts[name][:, 0:KH], start=False, stop=False)
                        nc.tensor.matmul(ps[:], xb[1][:, pb * P:(pb + 1) * P],
                                         wts[name][:, KH:2 * KH], start=False, stop=True)
                        nc.scalar.copy(dests[name][:, pb * KH:(pb + 1) * KH], ps[:])

                if stages < 2:
                    continue

                # ---- attention, per quarter (2 pos-blocks) ----
                for qt in range(NQ):
                    g0 = 16 * qt
                    fs = 1024 * qt           # free-dim start in [128, 4096] tiles
                    qsl = slice(fs, fs + 1024)
                    # scores (gpsimd streaming measured 3-5x slower than
                    # DVE + port contention, so everything stays on DVE)
                    for t in range(R):
                        pr = sprod.tile([P, 1024], bf16, name="sprodt", tag="sp")
                        nc.vector.tensor_mul(pr[:], qbt[:, qsl], kbt[t][:, qsl])
                        nc.vector.tensor_reduce(
                            out=S3[:, g0:g0 + 16, t],
                            in_=pr[:].rearrange("p (g d) -> p g d", d=KD),
                            axis=AX.X, op=OP.add)
                    nc.vector.memset(S3[:, g0:g0 + 16, R:R + 1], 0)  # zero key

                    if stages < 3:
                        continue
                    # softmax over T slots. No max-subtraction: scores are
                    # O(1) (weights are 0.02-scale, q pre-divided by temp),
                    # softmax is shift-invariant, f32 exp can't overflow.
                    zsq = zs[:, g0:g0 + 16]
                    nc.scalar.activation(at3[:, g0:g0 + 16, 0:T],
                                         S3[:, g0:g0 + 16, 0:T], ACTF.Exp)
                    if T <= TOPK:
                        nc.vector.tensor_reduce(out=zsq, in_=at3[:, g0:g0 + 16, 0:T],
                                                axis=AX.X, op=OP.add)
                        nc.vector.reciprocal(zsq, zsq)
                        nc.vector.tensor_tensor(
                            at3[:, g0:g0 + 16, 0:T], at3[:, g0:g0 + 16, 0:T],
                            zsq.unsqueeze(2).broadcast_to([P, 16, T]), OP.mult)
                    else:
                        # T=5 sparse top-k on the unnormalized exps (the
                        # topk renorm makes the first softmax normalization
                        # redundant up to an EPS*Z ~ 5e-7 shift).
                        # delta = 4th-largest = 2nd-smallest of 5, via a
                        # 10-op min/max network.
                        a = [at3[:, g0:g0 + 16, i] for i in range(T)]
                        u1 = dmin[:, g0:g0 + 16]
                        u2 = mxp[:, g0:g0 + 16]
                        u3 = mx[:, g0:g0 + 16]
                        u4 = mx2[:, g0:g0 + 16]
                        nc.vector.tensor_tensor(u1, a[0], a[1], OP.min)   # m1
                        nc.vector.tensor_tensor(u2, a[0], a[1], OP.max)   # M1
                        nc.vector.tensor_tensor(u3, a[2], a[3], OP.min)   # m2
                        nc.vector.tensor_tensor(u4, a[2], a[3], OP.max)   # M2
                        nc.vector.tensor_tensor(u2, u2, u4, OP.min)       # c
                        nc.vector.tensor_tensor(u4, u1, u3, OP.max)       # s3
                        nc.vector.tensor_tensor(u1, u1, u3, OP.min)       # m3
                        nc.vector.tensor_tensor(u2, u4, u2, OP.min)       # 2nd of 4
                        nc.vector.tensor_tensor(u3, u1, a[4], OP.max)
                        nc.vector.tensor_tensor(u1, u3, u2, OP.min)       # delta
                        nc.vector.tensor_scalar_add(u1, u1, EPS)
                        nc.vector.tensor_tensor(
                            at3[:, g0:g0 + 16, 0:T], at3[:, g0:g0 + 16, 0:T],
                            u1.unsqueeze(2).broadcast_to([P, 16, T]), OP.subtract)
                        nc.vector.tensor_scalar_max(at3[:, g0:g0 + 16, 0:T],
                                                    at3[:, g0:g0 + 16, 0:T], 0.0)
                        nc.vector.tensor_reduce(out=zsq, in_=at3[:, g0:g0 + 16, 0:T],
                                                axis=AX.X, op=OP.add)
                        nc.vector.tensor_scalar_add(zsq, zsq, EPS)
                        nc.vector.reciprocal(zsq, zsq)
                        nc.vector.tensor_tensor(
                            at3[:, g0:g0 + 16, 0:T], at3[:, g0:g0 + 16, 0:T],
                            zsq.unsqueeze(2).broadcast_to([P, 16, T]), OP.mult)

                    nc.scalar.copy(attnb[:, 80 * qt:80 * qt + 80],
                                   attn[:, 80 * qt:80 * qt + 80])

                    if stages < 4:
                        continue
                    # weighted sum: o_q = sum_t attn_t * v_t  (bf16 pair tree)
                    oq = o[:, qsl].rearrange("p (g d) -> p g d", d=KD)

                    def wprod(dst3, t):
                        nc.vector.tensor_tensor(
                            dst3, vbt[t][:, qsl].rearrange("p (g d) -> p g d", d=KD),
                            ab3[:, g0:g0 + 16, t].unsqueeze(2).broadcast_to(
                                [P, 16, KD]), OP.mult)

                    if R == 1:
                        wprod(oq, 0)
                    else:
                        pts = []
                        for t in range(R):
                            pt = prodp.tile([P, 1024], bf16, name="wprod", tag="wp")
                            wprod(pt[:].rearrange("p (g d) -> p g d", d=KD), t)
                            pts.append(pt)
                        if R == 2:
                            nc.vector.tensor_add(o[:, qsl], pts[0][:], pts[1][:])
                        elif R == 3:
                            t01 = prodp.tile([P, 1024], bf16, name="wprod", tag="wp")
                            nc.vector.tensor_add(t01[:], pts[0][:], pts[1][:])
                            nc.vector.tensor_add(o[:, qsl], t01[:], pts[2][:])
                        else:
                            t01 = prodp.tile([P, 1024], bf16, name="wprod", tag="wp")
                            t23 = prodp.tile([P, 1024], bf16, name="wprod", tag="wp")
                            nc.vector.tensor_add(t01[:], pts[0][:], pts[1][:])
                            nc.vector.tensor_add(t23[:], pts[2][:], pts[3][:])
                            nc.vector.tensor_add(o[:, qsl], t01[:], t23[:])

                    if stages < 5:
                        continue
                    # PE transpose o -> channel-major opad for this quarter
                    for pb in (2 * qt, 2 * qt + 1):
                        for ht in range(4):
                            tps = xps.tile([P, 1024], bf16, name="xpose_ps")
                            nc.tensor.transpose(
                                tps[:, 0:P],
                                o[:, pb * KH + ht * P:pb * KH + (ht + 1) * P],
                                ident[:])
                            opv = opad[ht][:, 0:PHW].rearrange(
                                "c (i j) -> c i j", j=PW)
                            nc.scalar.copy(
                                opv[:, 1 + 4 * pb:5 + 4 * pb, 1:W + 1],
                                tps[:, 0:P].rearrange("c (i j) -> c i j", j=W))

                if stages < 6:
                    continue
                # ---- conv3x3 #1: y1 = W1 * opad, 8-row chunks ----
                # weight-stationary over chunk pairs: both chunks of a
                # half run back-to-back per weight tile so LDWEIGHTS is
                # shared; half granularity keeps the attention overlap.
                wl1 = w1t[l % 2]
                for half in range(2):
                    for co in range(2):
                        pss = [convps.tile([P, 512], f32, name="c1ps", tag="cps")
                               for _ in range(2)]
                        for tap in range(9):
                            ty, tx = tap // 3, tap % 3
                            for ci in range(4):
                                f = (tap * 4 + ci) * 2 + co
                                for ic in range(2):
                                    i0 = (half * 2 + ic) * RPC
                                    base = PW * (i0 + ty) + tx
                                    nc.tensor.matmul(
                                        pss[ic][:, 0:CW], wl1[:, f * P:(f + 1) * P],
                                        opad[ci][:, base:base + CW],
                                        start=(tap == 0 and ci == 0),
                                        stop=(tap == 8 and ci == 3))
                        for ic in range(2):
                            c = half * 2 + ic
                            i0 = c * RPC
                            nc.scalar.copy(
                                y1[co][:, W * i0:W * (i0 + RPC)].rearrange(
                                    "c (i j) -> c i j", j=W),
                                pss[ic][:, 0:CW].rearrange(
                                    "c (i j) -> c i j", j=PW)[:, :, 0:W])
                            # per-chunk BN stats while DVE is otherwise idle
                            # st columns: (co*2 + kind)*4 + chunk
                            ysl = y1[co][:, W * i0:W * (i0 + RPC)]
                            i_sum = (co * 2 + 0) * 4 + c
                            i_sq = (co * 2 + 1) * 4 + c
                            nc.vector.tensor_reduce(out=st[:, i_sum:i_sum + 1],
                                                    in_=ysl, axis=AX.X, op=OP.add)
                            sqt = sprod.tile([P, 1024], bf16, name="sqt", tag="sp")
                            nc.scalar.square(sqt[:, 0:W * RPC], ysl)
                            nc.vector.tensor_reduce(out=st[:, i_sq:i_sq + 1],
                                                    in_=sqt[:, 0:W * RPC],
                                                    axis=AX.X, op=OP.add)

                if stages < 7:
                    continue
                # ---- BN stats AllGather ----
                # PE-transpose the [128,4] stats to [4,128] so both DMA
                # legs move contiguous 512B rows (the naive layouts cost
                # ~8us each in 16B/4B scattered packets).
                nc.vector.tensor_reduce(
                    out=st2[:], in_=st[:].rearrange("p (g c) -> p g c", c=4),
                    axis=AX.X, op=OP.add)
                tp0 = kqvps.tile([4, P], f32, name="stT_ps", tag="kqv_ps")
                nc.tensor.transpose(tp0[:], st2[:], identf[:])
                nc.scalar.copy(stT[:], tp0[:])
                cci = dramp.tile([4, P], f32, name="cci")
                cco = dramp.tile([ncores, 512], f32, name="cco", addr_space="Shared")
                nc.sync.dma_start(cci[:], stT[:])
                nc.gpsimd.collective_compute(
                    "AllGather", OP.bypass,
                    replica_groups=[list(range(ncores))],
                    ins=[cci.opt()], outs=[cco.opt()])
                nc.sync.dma_start(gst8[:], cco[:])
                # transpose each [8,128] kind-block back and reduce over cores
                tpb = kqvps.tile([P, 32], f32, name="gsum_ps", tag="kqv_ps")
                for k in range(4):
                    nc.tensor.transpose(tpb[:, 8 * k:8 * (k + 1)],
                                        gst8[:, P * k:P * (k + 1)],
                                        identf[0:NC, 0:NC])
                    nc.vector.tensor_reduce(
                        out=gsum[:, k:k + 1], in_=tpb[:, 8 * k:8 * (k + 1)],
                        axis=AX.X, op=OP.add)

                # prefetch next layer's conv weights (overlaps conv2 + next KQV)
                if l + 1 < layers:
                    nc.sync.dma_start(w1t[(l + 1) % 2][:], w1d[l + 1])
                    nc.sync.dma_start(w2t[(l + 1) % 2][:], w2d[l + 1])

                if stages < 8:
                    continue
                # ---- BN coefficients: A = g/sqrt(var+eps), B = b - mean*A ----
                # computed for both co halves at once on [P,2] tiles
                NTOT = float(ncores * HW)
                g4 = gsum[:].rearrange("p (co k) -> p co k", k=2)
                nc.vector.tensor_scalar_mul(mean2[:], g4[:, :, 0], 1.0 / NTOT)
                nc.vector.tensor_scalar_mul(vart[:], g4[:, :, 1], 1.0 / NTOT)
                nc.vector.tensor_mul(sq2[:], mean2[:], mean2[:])
                nc.vector.tensor_sub(vart[:], vart[:], sq2[:])
                nc.vector.tensor_scalar_add(vart[:], vart[:], BN_EPS)
                nc.scalar.activation(stdt[:], vart[:], ACTF.Sqrt)
                nc.vector.reciprocal(stdt[:], stdt[:])
                nc.vector.tensor_mul(A2[:], bngt[:], stdt[:])
                nc.vector.tensor_mul(sq2[:], mean2[:], A2[:])
                nc.vector.tensor_sub(B2[:], bnbt[:], sq2[:])
                # h1 = relu(A*y1 + B), strided bf16 into padded conv2 input;
                # split in row halves so conv2's first chunks start sooner
                for rh in range(2):
                    r0 = rh * (H // 2)
                    for co in range(2):
                        h1v = h1p[co][:, 0:PHW].rearrange("c (i j) -> c i j", j=PW)
                        y1v = y1[co][:].rearrange("c (i j) -> c i j", j=W)
                        nc.scalar.activation(
                            h1v[:, 1 + r0:1 + r0 + H // 2, 1:W + 1],
                            y1v[:, r0:r0 + H // 2, :],
                            ACTF.Relu, bias=B2[:, co:co + 1], scale=A2[:, co:co + 1])

                if stages < 9:
                    continue
                # ---- conv3x3 #2 + residual x += gamma*(h2 + ob2) ----
                wl2 = w2t[l % 2]
                for co in range(2):
                    nc.scalar.add(x[co][:], x[co][:], gob2t[:, co:co + 1])
                for half in range(2):
                    for co in range(2):
                        pss = [convps.tile([P, 512], f32, name="c2ps", tag="cps")
                               for _ in range(2)]
                        for tap in range(9):
                            ty, tx = tap // 3, tap % 3
                            for ci in range(2):
                                f = (tap * 2 + ci) * 2 + co
                                for ic in range(2):
                                    i0 = (half * 2 + ic) * RPC
                                    base = PW * (i0 + ty) + tx
                                    nc.tensor.matmul(
                                        pss[ic][:, 0:CW], wl2[:, f * P:(f + 1) * P],
                                        h1p[ci][:, base:base + CW],
                                        start=(tap == 0 and ci == 0),
                                        stop=(tap == 8 and ci == 1))
                        for ic in range(2):
                            i0 = (half * 2 + ic) * RPC
                            xslice = x[co][:, W * i0:W * (i0 + RPC)]
                            nc.vector.scalar_tensor_tensor(
                                out=xslice.rearrange("c (i j) -> c i j", j=W),
                                in0=pss[ic][:, 0:CW].rearrange(
                                    "c (i j) -> c i j", j=PW)[:, :, 0:W],
                                scalar=gamt[:],
                                in1=xslice.rearrange("c (i j) -> c i j", j=W),
                                op0=OP.mult, op1=OP.add)
                        if l == layers - 1:
                            # stream the finished half straight out
                            if co == 0 and half == 0:
                                # consume the warmup-AllGather / warmup-
                                # matmul zeros (adds 0.0; anti-DCE)
                                nc.vector.tensor_tensor(
                                    x[0][0:1, 0:8], x[0][0:1, 0:8],
                                    wres[:], OP.add)
                                nc.vector.tensor_tensor(
                                    x[0][0:1, 0:8], x[0][0:1, 0:8],
                                    wscr[0:1, 0:8], OP.add)
                            hs = slice(512 * half, 512 * (half + 1))
                            nc.sync.dma_start(out[co * P:(co + 1) * P, hs],
                                              x[co][:, hs])
                if l < layers - 1:
                    for co in range(2):
                        nc.scalar.copy(xb[co][:], x[co][:])

    nc.compile()
    return nc


def _host_prep(inputs):
    bf = ml_dtypes.bfloat16
    kw, kb, qw, qb = inputs["kw"], inputs["kb"], inputs["qw"], inputs["qb"]
    vw, vb = inputs["vw"], inputs["vb"]
    ow1, ow2 = inputs["ow1"], inputs["ow2"]
    gammas, ob2 = inputs["gammas"], inputs["ob2"]

    def packw(wm):  # [L, KH, C] -> [L, 128, 2*KH]  (lhsT per ci-half, fused)
        a = wm.transpose(0, 2, 1).reshape(L, 2, P, KH)   # [L, ci, cin128, KH]
        return np.ascontiguousarray(a.transpose(0, 2, 1, 3).reshape(L, P, 2 * KH)
                                    ).astype(bf)

    d = {}
    d["wq"] = packw(qw / 8.0)
    d["wk"] = packw(kw)
    d["wv"] = packw(vw)
    d["bq"] = np.ascontiguousarray((qb / 8.0).reshape(L, 1, KH)).astype(bf)
    d["bk"] = np.ascontiguousarray(kb.reshape(L, 1, KH)).astype(bf)
    d["bv"] = np.ascontiguousarray(vb.reshape(L, 1, KH)).astype(bf)
    # ow1 [L, 256, 512, 3, 3] -> [L, cin128, f=(tap, ci4, co2), cout128]
    a1 = ow1.reshape(L, 2, P, 4, P, 3, 3)   # [L, co, cout, ci, cin, ty, tx]
    a1 = a1.transpose(0, 4, 5, 6, 3, 1, 2)  # [L, cin, ty, tx, ci, co, cout]
    d["w1d"] = np.ascontiguousarray(a1.reshape(L, P, 72 * P)).astype(bf)
    a2 = ow2.reshape(L, 2, P, 2, P, 3, 3)
    a2 = a2.transpose(0, 4, 5, 6, 3, 1, 2)  # [L, cin, ty, tx, ci, co, cout]
    d["w2d"] = np.ascontiguousarray(a2.reshape(L, P, 36 * P)).astype(bf)
    d["bngd"] = np.ascontiguousarray(
        inputs["bn_g"].reshape(L, 2, P).transpose(0, 2, 1)).astype(np.float32)
    d["bnbd"] = np.ascontiguousarray(
        inputs["bn_b"].reshape(L, 2, P).transpose(0, 2, 1)).astype(np.float32)
    gob2 = gammas[:, None] * ob2
    d["gob2d"] = np.ascontiguousarray(
        gob2.reshape(L, 2, P).transpose(0, 2, 1)).astype(np.float32)
    d["gamd"] = np.ascontiguousarray(
        np.broadcast_to(gammas[:, None, None], (L, P, 1))).astype(np.float32)
    d["identd"] = np.eye(P, dtype=np.float32).astype(bf)
    return d


def kernel(**inputs):
    if "nc" not in _compiled:
        _compiled["nc"] = _build()
    nc = _compiled["nc"]
    shared = _host_prep(inputs)
    x = np.ascontiguousarray(inputs["x"].reshape(B, C, HW)).astype(np.float32)
    in_maps = []
    for c in range(NC):
        m = dict(shared)
        m["xin"] = x[c]
        in_maps.append(m)
    res = bass_utils.run_bass_kernel_spmd(nc, in_maps, core_ids=list(range(NC)))
    outs = np.stack([res.results[c]["out"] for c in range(NC)])
    return outs.reshape(B, C, H, W).astype(np.float32)


# revision 53
# speedup vs baseline: 1.0215x; 1.0215x over previous
"""AttentiveDensenet Trainium2 Bass kernel (v2).

Data-parallel over batch B=8 across 8 NeuronCores (1 image per core).

v2 changes vs v1 (driven by trace analysis of the 1.52ms baseline):
  - Conv weights are host-packed so each layer's w1/w2 load as ONE big
    DMA each into resident SBUF tiles (double-buffered, prefetched a
    layer ahead). Kills the per-tile weight DMA storm (1392 SP DMA
    issues, 299k tiny packets) that starved the PE and kept the HAM
    clock gate cold (PE ran at 1.2 GHz for 87% of the run).
  - o is transposed to channel-major with PE transpose-mode matmuls
    (32x [128,128] blocks/layer) + ACT evac into the padded conv input,
    replacing the DRAM bounce + xbar-transpose chain (~40us/layer of
    dead time).
  - Attention is emitted per quarter-image (2 pos-blocks) and conv1 in
    8-row chunks interleaved with the o-transposes, so DVE attention
    overlaps PE conv work instead of serializing.
  - Weighted sum accumulates bf16 products with a pair tree (2x DVE
    mode) instead of sequential f32 adds.
  - A dummy AllGather at kernel start absorbs the ~47us core-launch
    skew barrier under layer-0 compute; per-layer BN stat AllGathers
    then run near their intrinsic latency.
  - BN stats are reduced per conv1-chunk as results land (DVE is idle
    then), so the AllGather starts immediately after the last chunk.
"""
import numpy as np
import ml_dtypes

import concourse.bacc as bacc
import concourse.mybir as mybir
import concourse.tile as tile
from concourse import bass_utils

L, C, B, H, W = 4, 256, 8, 32, 32
NH, KD = 8, 64
KH = NH * KD          # 512
HW = H * W            # 1024
P = 128
NC = 8                # cores
TOPK = 4
EPS = 1e-7
BN_EPS = 1e-5
PW = W + 2            # 34
PHW = PW * (H + 2)    # 1156
NQ = 4                # quarters (2 pos-blocks each)
RPC = 8               # conv chunk rows
NCH = H // RPC        # 4 conv chunks
CW = PW * RPC         # 272 conv chunk width (incl pad cols)

f32 = mybir.dt.float32
bf16 = mybir.dt.bfloat16
AX = mybir.AxisListType
OP = mybir.AluOpType
ACTF = mybir.ActivationFunctionType

_compiled = {}


def _build(ncores=NC, layers=L, stages=99):
    nc = bacc.Bacc(None, target_bir_lowering=False, debug=False, num_devices=ncores)

    # ---- DRAM I/O (per-core shapes; weights replicated) ----
    xin = nc.dram_tensor("xin", [C, HW], f32, kind="ExternalInput").ap()
    wq = nc.dram_tensor("wq", [L, P, 2 * KH], bf16, kind="ExternalInput").ap()
    wk = nc.dram_tensor("wk", [L, P, 2 * KH], bf16, kind="ExternalInput").ap()
    wv = nc.dram_tensor("wv", [L, P, 2 * KH], bf16, kind="ExternalInput").ap()
    bq = nc.dram_tensor("bq", [L, 1, KH], bf16, kind="ExternalInput").ap()
    bk = nc.dram_tensor("bk", [L, 1, KH], bf16, kind="ExternalInput").ap()
    bv = nc.dram_tensor("bv", [L, 1, KH], bf16, kind="ExternalInput").ap()
    w1d = nc.dram_tensor("w1d", [L, P, 72 * P], bf16, kind="ExternalInput").ap()
    w2d = nc.dram_tensor("w2d", [L, P, 36 * P], bf16, kind="ExternalInput").ap()
    bngd = nc.dram_tensor("bngd", [L, P, 2], f32, kind="ExternalInput").ap()
    bnbd = nc.dram_tensor("bnbd", [L, P, 2], f32, kind="ExternalInput").ap()
    gob2d = nc.dram_tensor("gob2d", [L, P, 2], f32, kind="ExternalInput").ap()
    gamd = nc.dram_tensor("gamd", [L, P, 1], f32, kind="ExternalInput").ap()
    identd = nc.dram_tensor("identd", [P, P], bf16, kind="ExternalInput").ap()
    out = nc.dram_tensor("out", [C, HW], f32, kind="ExternalOutput").ap()

    with tile.TileContext(nc) as tc:
        with tc.tile_pool(name="main", bufs=1) as mp, \
             tc.tile_pool(name="prodp", bufs=6) as prodp, \
             tc.tile_pool(name="sprod", bufs=2) as sprod, \
             tc.tile_pool(name="wkvp", bufs=4) as wkvp, \
             tc.tile_pool(name="biasp", bufs=3) as biasp, \
             tc.tile_pool(name="kqvps", bufs=2, space="PSUM") as kqvps, \
             tc.tile_pool(name="convps", bufs=4, space="PSUM") as convps, \
             tc.tile_pool(name="xps", bufs=2, space="PSUM") as xps, \
             tc.tile_pool(name="dramp", bufs=2, space="DRAM") as dramp:

            # ---- persistent tiles ----
            x = [mp.tile([P, HW], f32, name=f"x{i}") for i in range(2)]
            xb = [mp.tile([P, HW], bf16, name=f"xb{i}") for i in range(2)]
            qbt = mp.tile([P, 8 * KH], bf16, name="qbt")
            kbt = [mp.tile([P, 8 * KH], bf16, name=f"kbt{i}") for i in range(L)]
            vbt = [mp.tile([P, 8 * KH], bf16, name=f"vbt{i}") for i in range(L)]
            S = mp.tile([P, 64 * 5], f32, name="S")
            attn = mp.tile([P, 64 * 5], f32, name="attn")
            attnb = mp.tile([P, 64 * 5], bf16, name="attnb")
            mx = mp.tile([P, 64], f32, name="mx")
            mx2 = mp.tile([P, 64], f32, name="mx2")
            zs = mp.tile([P, 64], f32, name="zs")
            dmin = mp.tile([P, 64], f32, name="dmin")
            mxp = mp.tile([P, 64], f32, name="mxp")
            o = mp.tile([P, 8 * KH], bf16, name="o")
            opad = [mp.tile([P, PHW + 2], bf16, name=f"opad{i}") for i in range(4)]
            y1 = [mp.tile([P, HW], f32, name=f"y1_{i}") for i in range(2)]
            h1p = [mp.tile([P, PHW + 2], bf16, name=f"h1p{i}") for i in range(2)]
            st = mp.tile([P, 16], f32, name="st")       # (co, kind, chunk)
            st2 = mp.tile([P, 4], f32, name="st2")      # (co, kind)
            stT = mp.tile([4, P], f32, name="stT")      # transposed stats
            gst8 = mp.tile([NC, 512], f32, name="gst8")  # gathered rows
            gsum = mp.tile([P, 4], f32, name="gsum")
            ones1 = mp.tile([1, P], bf16, name="ones1")
            ident = mp.tile([P, P], bf16, name="ident")
            identf = mp.tile([P, P], f32, name="identf")
            wres = mp.tile([1, 8], f32, name="wres")
            wsrc = mp.tile([1, 8], f32, name="wsrc")
            # resident conv weights, double-buffered across layers
            w1t = [mp.tile([P, 72 * P], bf16, name=f"w1t{i}") for i in range(2)]
            w2t = [mp.tile([P, 36 * P], bf16, name=f"w2t{i}") for i in range(2)]
            # per-layer consts (columns = co half)
            bngt = mp.tile([P, 2], f32, name="bngt")
            bnbt = mp.tile([P, 2], f32, name="bnbt")
            gob2t = mp.tile([P, 2], f32, name="gob2t")
            gamt = mp.tile([P, 1], f32, name="gamt")
            # BN scratch (columns = co half)
            mean2 = mp.tile([P, 2], f32, name="mean2")
            vart = mp.tile([P, 2], f32, name="vart")
            sq2 = mp.tile([P, 2], f32, name="sq2")
            stdt = mp.tile([P, 2], f32, name="stdt")
            A2 = mp.tile([P, 2], f32, name="A2")
            B2 = mp.tile([P, 2], f32, name="B2")

            # ---- init ----
            for i in range(2):
                nc.sync.dma_start(x[i][:], xin[i * P:(i + 1) * P, :])
                nc.scalar.copy(xb[i][:], x[i][:])
            nc.sync.dma_start(ident[:], identd)
            nc.vector.tensor_copy(identf[:], ident[:])
            for i in range(4):
                nc.vector.memset(opad[i][:], 0)
            for i in range(2):
                nc.vector.memset(h1p[i][:], 0)
            nc.vector.memset(ones1[:], 1.0)
            nc.vector.memset(S[:], 0)
            nc.vector.memset(attn[:], 0)
            nc.vector.memset(attnb[:], 0)
            nc.vector.memset(wsrc[:], 0)

            # PE warmup: ~5us of dummy matmuls (zeros) so the HAM clock
            # gate reaches 2.4 GHz before the first real KQV matmul.
            # Their (zero) result is consumed additively at the end.
            wscr = mp.tile([1, 512], bf16, name="wscr")
            for wi in range(10):
                wps = kqvps.tile([P, KH], f32, name="kqv_ps")
                nc.tensor.matmul(wps[:, 0:320], ones1[:], attnb[0:1, 0:320],
                                 start=True, stop=False)
                nc.tensor.matmul(wps[:, 0:320], ones1[:], attnb[0:1, 0:320],
                                 start=False, stop=True)
                if wi == 9:
                    nc.scalar.copy(wscr[0:1, 0:320], wps[0:1, 0:320])

            # warmup collective: absorbs core-launch skew + CC init
            # barrier while layer-0 compute runs. wres (all zeros) is
            # consumed additively just before the output DMA so DCE
            # can't drop the chain.
            wrmi = dramp.tile([1, 8], f32, name="wrmi")
            wrmo = dramp.tile([ncores, 8], f32, name="wrmo", addr_space="Shared")
            nc.sync.dma_start(wrmi[:], wsrc[:])
            nc.gpsimd.collective_compute(
                "AllGather", OP.bypass,
                replica_groups=[list(range(ncores))],
                ins=[wrmi.opt()], outs=[wrmo.opt()])
            nc.sync.dma_start(wres[:], wrmo[0:1, :])



            S3 = S[:].rearrange("p (g t) -> p g t", t=5)
            at3 = attn[:].rearrange("p (g t) -> p g t", t=5)
            ab3 = attnb[:].rearrange("p (g t) -> p g t", t=5)


            for l in range(layers):
                R = l + 1      # number of real keys
                T = R + 1      # +1 zero key

                # ---- per-layer consts ----
                nc.sync.dma_start(bngt[:], bngd[l])
                nc.sync.dma_start(bnbt[:], bnbd[l])
                nc.sync.dma_start(gob2t[:], gob2d[l])
                nc.sync.dma_start(gamt[:], gamd[l])

                # ---- K/Q/V 1x1 convs, position-major ----
                wts, bts = {}, {}
                for name, wdr, bdr in (("k", wk, bk), ("v", wv, bv), ("q", wq, bq)):
                    bt = biasp.tile([1, KH], bf16, name=f"bias_{name}", tag="bias")
                    nc.sync.dma_start(bt[:], bdr[l])
                    wt = wkvp.tile([P, 2 * KH], bf16, name=f"w_{name}", tag="wkv")
                    nc.sync.dma_start(wt[:], wdr[l])
                    wts[name], bts[name] = wt, bt
                if l == 0:
                    # layer-0 conv weights (after the KQV weight DMAs so
                    # those win the queue race; conv1 needs these ~40us in)
                    nc.sync.dma_start(w1t[0][:], w1d[0])
                    nc.sync.dma_start(w2t[0][:], w2d[0])
                dests = {"k": kbt[l][:], "v": vbt[l][:], "q": qbt[:]}
                for pb in range(8):
                    for name in ("k", "v", "q"):
                        ps = kqvps.tile([P, KH], f32, name="kqv_ps")
                        nc.tensor.matmul(ps[:], ones1[:], bts[name][:],
                                         start=True, stop=False)
                        nc.tensor.matmul(ps[:], xb[0][:, pb * P:(pb + 1) * P],
                                         wts[name][:, 0:KH], start=False, stop=False)
                        nc.tensor.matmul(ps[:], xb[1][:, pb * P:(pb + 1) * P],
                                         wts[name][:, KH:2 * KH], start=False, stop=True)
                        nc.scalar.copy(dests[name][:, pb * KH:(pb + 1) * KH], ps[:])

                if stages < 2:
                    continue

                # ---- attention, per quarter (2 pos-blocks) ----
                for qt in range(NQ):
                    g0 = 16 * qt
                    fs = 1024 * qt           # free-dim start in [128, 4096] tiles
                    qsl = slice(fs, fs + 1024)
                    # scores (gpsimd streaming measured 3-5x slower than
                    # DVE + port contention, so everything stays on DVE)
                    for t in range(R):
                        pr = sprod.tile([P, 1024], bf16, name="sprodt", tag="sp")
                        nc.vector.tensor_mul(pr[:], qbt[:, qsl], kbt[t][:, qsl])
                        nc.vector.tensor_reduce(
                            out=S3[:, g0:g0 + 16, t],
                            in_=pr[:].rearrange("p (g d) -> p g d", d=KD),
                            axis=AX.X, op=OP.add)
                    nc.vector.memset(S3[:, g0:g0 + 16, R:R + 1], 0)  # zero key

                    if stages < 3:
                        continue
                    # softmax over T slots. No max-subtraction: scores are
                    # O(1) (weights are 0.02-scale, q pre-divided by temp),
                    # softmax is shift-invariant, f32 exp can't overflow.
                    zsq = zs[:, g0:g0 + 16]
                    nc.scalar.activation(at3[:, g0:g0 + 16, 0:T],
                                         S3[:, g0:g0 + 16, 0:T], ACTF.Exp)
                    if T <= TOPK:
                        nc.vector.tensor_reduce(out=zsq, in_=at3[:, g0:g0 + 16, 0:T],
                                                axis=AX.X, op=OP.add)
                        nc.vector.reciprocal(zsq, zsq)
                        nc.vector.tensor_tensor(
                            at3[:, g0:g0 + 16, 0:T], at3[:, g0:g0 + 16, 0:T],
                            zsq.unsqueeze(2).broadcast_to([P, 16, T]), OP.mult)
                    else:
                        # T=5 sparse top-k on the unnormalized exps (the
                        # topk renorm makes the first softmax normalization
                        # redundant up to an EPS*Z ~ 5e-7 shift).
                        # delta = 4th-largest = 2nd-smallest of 5, via a
                        # 10-op min/max network.
                        a = [at3[:, g0:g0 + 16, i] for i in range(T)]
                        u1 = dmin[:, g0:g0 + 16]
                        u2 = mxp[:, g0:g0 + 16]
                        u3 = mx[:, g0:g0 + 16]
                        u4 = mx2[:, g0:g0 + 16]
                        nc.vector.tensor_tensor(u1, a[0], a[1], OP.min)   # m1
                        nc.vector.tensor_tensor(u2, a[0], a[1], OP.max)   # M1
                        nc.vector.tensor_tensor(u3, a[2], a[3], OP.min)   # m2
                        nc.vector.tensor_tensor(u4, a[2], a[3], OP.max)   # M2
                        nc.vector.tensor_tensor(u2, u2, u4, OP.min)       # c
                        nc.vector.tensor_tensor(u4, u1, u3, OP.max)       # s3
                        nc.vector.tensor_tensor(u1, u1, u3, OP.min)       # m3
                        nc.vector.tensor_tensor(u2, u4, u2, OP.min)       # 2nd of 4
                        nc.vector.tensor_tensor(u3, u1, a[4], OP.max)
                        nc.vector.tensor_tensor(u1, u3, u2, OP.min)       # delta
                        nc.vector.tensor_scalar_add(u1, u1, EPS)
                        nc.vector.tensor_tensor(
                            at3[:, g0:g0 + 16, 0:T], at3[:, g0:g0 + 16, 0:T],
                            u1.unsqueeze(2).broadcast_to([P, 16, T]), OP.subtract)
                        nc.vector.tensor_scalar_max(at3[:, g0:g0 + 16, 0:T],
                                                    at3[:, g0:g0 + 16, 0:T], 0.0)
                        nc.vector.tensor_reduce(out=zsq, in_=at3[:, g0:g0 + 16, 0:T],
                                                axis=AX.X, op=OP.add)
                        nc.vector.tensor_scalar_add(zsq, zsq, EPS)
                        nc.vector.reciprocal(zsq, zsq)
                        nc.vector.tensor_tensor(
                            at3[:, g0:g0 + 16, 0:T], at3[:, g0:g0 + 16, 0:T],
                            zsq.unsqueeze(2).broadcast_to([P, 16, T]), OP.mult)

                    nc.scalar.copy(attnb[:, 80 * qt:80 * qt + 80],
                                   attn[:, 80 * qt:80 * qt + 80])

                    if stages < 4:
                        continue
                    # weighted sum: o_q = sum_t attn_t * v_t  (bf16 pair tree)
                    oq = o[:, qsl].rearrange("p (g d) -> p g d", d=KD)

                    def wprod(dst3, t):
                        nc.vector.tensor_tensor(
                            dst3, vbt[t][:, qsl].rearrange("p (g d) -> p g d", d=KD),
                            ab3[:, g0:g0 + 16, t].unsqueeze(2).broadcast_to(
                                [P, 16, KD]), OP.mult)

                    if R == 1:
                        wprod(oq, 0)
                    else:
                        pts = []
                        for t in range(R):
                            pt = prodp.tile([P, 1024], bf16, name="wprod", tag="wp")
                            wprod(pt[:].rearrange("p (g d) -> p g d", d=KD), t)
                            pts.append(pt)
                        if R == 2:
                            nc.vector.tensor_add(o[:, qsl], pts[0][:], pts[1][:])
                        elif R == 3:
                            t01 = prodp.tile([P, 1024], bf16, name="wprod", tag="wp")
                            nc.vector.tensor_add(t01[:], pts[0][:], pts[1][:])
                            nc.vector.tensor_add(o[:, qsl], t01[:], pts[2][:])
                        else:
                            t01 = prodp.tile([P, 1024], bf16, name="wprod", tag="wp")
                            t23 = prodp.tile([P, 1024], bf16, name="wprod", tag="wp")
                            nc.vector.tensor_add(t01[:], pts[0][:], pts[1][:])
                            nc.vector.tensor_add(t23[:], pts[2][:], pts[3][:])
                            nc.vector.tensor_add(o[:, qsl], t01[:], t23[:])

                    if stages < 5:
                        continue
                    # PE transpose o -> channel-major opad for this quarter
                    for pb in (2 * qt, 2 * qt + 1):
                        for ht in range(4):
                            tps = xps.tile([P, 1024], bf16, name="xpose_ps")
                            nc.tensor.transpose(
                                tps[:, 0:P],
                                o[:, pb * KH + ht * P:pb * KH + (ht + 1) * P],
                                ident[:])
                            opv = opad[ht][:, 0:PHW].rearrange(
                                "c (i j) -> c i j", j=PW)
                            nc.scalar.copy(
                                opv[:, 1 + 4 * pb:5 + 4 * pb, 1:W + 1],
                                tps[:, 0:P].rearrange("c (i j) -> c i j", j=W))

                if stages < 6:
                    continue
                # ---- conv3x3 #1: y1 = W1 * opad, 8-row chunks ----
                # weight-stationary over chunk pairs: both chunks of a
                # half run back-to-back per weight tile so LDWEIGHTS is
                # shared; half granularity keeps the attention overlap.
                wl1 = w1t[l % 2]
                for half in range(2):
                    for co in range(2):
                        pss = [convps.tile([P, 512], f32, name="c1ps", tag="cps")
                               for _ in range(2)]
                        for tap in range(9):
                            ty, tx = tap // 3, tap % 3
                            for ci in range(4):
                                f = (tap * 4 + ci) * 2 + co
                                for ic in range(2):
                                    i0 = (half * 2 + ic) * RPC
                                    base = PW * (i0 + ty) + tx
                                    nc.tensor.matmul(
                                        pss[ic][:, 0:CW], wl1[:, f * P:(f + 1) * P],
                                        opad[ci][:, base:base + CW],
                                        start=(tap == 0 and ci == 0),
                                        stop=(tap == 8 and ci == 3))
                        for ic in range(2):
                            c = half * 2 + ic
                            i0 = c * RPC
                            nc.scalar.copy(
                                y1[co][:, W * i0:W * (i0 + RPC)].rearrange(
                                    "c (i j) -> c i j", j=W),
                                pss[ic][:, 0:CW].rearrange(
                                    "c (i j) -> c i j", j=PW)[:, :, 0:W])
                            # per-chunk BN stats while DVE is otherwise idle
                            # st columns: (co*2 + kind)*4 + chunk
                            ysl = y1[co][:, W * i0:W * (i0 + RPC)]
                            i_sum = (co * 2 + 0) * 4 + c
                            i_sq = (co * 2 + 1) * 4 + c
                            nc.vector.tensor_reduce(out=st[:, i_sum:i_sum + 1],
                                                    in_=ysl, axis=AX.X, op=OP.add)
                            sqt = sprod.tile([P, 1024], bf16, name="sqt", tag="sp")
                            nc.scalar.square(sqt[:, 0:W * RPC], ysl)
                            nc.vector.tensor_reduce(out=st[:, i_sq:i_sq + 1],
                                                    in_=sqt[:, 0:W * RPC],
                                                    axis=AX.X, op=OP.add)

                if stages < 7:
                    continue
                # ---- BN stats AllGather ----
                # PE-transpose the [128,4] stats to [4,128] so both DMA
                # legs move contiguous 512B rows (the naive layouts cost
                # ~8us each in 16B/4B scattered packets).
                nc.vector.tensor_reduce(
                    out=st2[:], in_=st[:].rearrange("p (g c) -> p g c", c=4),
                    axis=AX.X, op=OP.add)
                tp0 = kqvps.tile([4, P], f32, name="stT_ps", tag="kqv_ps")
                nc.tensor.transpose(tp0[:], st2[:], identf[:])
                nc.scalar.copy(stT[:], tp0[:])
                cci = dramp.tile([4, P], f32, name="cci")
                cco = dramp.tile([ncores, 512], f32, name="cco", addr_space="Shared")
                nc.sync.dma_start(cci[:], stT[:])
                nc.gpsimd.collective_compute(
                    "AllGather", OP.bypass,
                    replica_groups=[list(range(ncores))],
                    ins=[cci.opt()], outs=[cco.opt()])
                nc.sync.dma_start(gst8[:], cco[:])
                # transpose each [8,128] kind-block back and reduce over cores
                tpb = kqvps.tile([P, 32], f32, name="gsum_ps", tag="kqv_ps")
                for k in range(4):
                    nc.tensor.transpose(tpb[:, 8 * k:8 * (k + 1)],
                                        gst8[:, P * k:P * (k + 1)],
                                        identf[0:NC, 0:NC])
                    nc.vector.tensor_reduce(
                        out=gsum[:, k:k + 1], in_=tpb[:, 8 * k:8 * (k + 1)],
                        axis=AX.X, op=OP.add)

                # prefetch next layer's conv weights (overlaps conv2 + next KQV)
                if l + 1 < layers:
                    nc.sync.dma_start(w1t[(l + 1) % 2][:], w1d[l + 1])
                    nc.sync.dma_start(w2t[(l + 1) % 2][:], w2d[l + 1])

                if stages < 8:
                    continue
                # ---- BN coefficients: A = g/sqrt(var+eps), B = b - mean*A ----
                # computed for both co halves at once on [P,2] tiles
                NTOT = float(ncores * HW)
                g4 = gsum[:].rearrange("p (co k) -> p co k", k=2)
                nc.vector.tensor_scalar_mul(mean2[:], g4[:, :, 0], 1.0 / NTOT)
                nc.vector.tensor_scalar_mul(vart[:], g4[:, :, 1], 1.0 / NTOT)
                nc.vector.tensor_mul(sq2[:], mean2[:], mean2[:])
                nc.vector.tensor_sub(vart[:], vart[:], sq2[:])
                nc.vector.tensor_scalar_add(vart[:], vart[:], BN_EPS)
                nc.scalar.activation(stdt[:], vart[:], ACTF.Sqrt)
                nc.vector.reciprocal(stdt[:], stdt[:])
                nc.vector.tensor_mul(A2[:], bngt[:], stdt[:])
                nc.vector.tensor_mul(sq2[:], mean2[:], A2[:])
                nc.vector.tensor_sub(B2[:], bnbt[:], sq2[:])
                # h1 = relu(A*y1 + B), strided bf16 into padded conv2 input;
                # split in row halves so conv2's first chunks start sooner
                for rh in range(2):
                    r0 = rh * (H // 2)
                    for co in range(2):
                        h1v = h1p[co][:, 0:PHW].rearrange("c (i j) -> c i j", j=PW)
                        y1v = y1[co][:].rearrange("c (i j) -> c i j", j=W)
                        nc.scalar.activation(
                            h1v[:, 1 + r0:1 + r0 + H // 2, 1:W + 1],
                            y1v[:, r0:r0 + H // 2, :],
                            ACTF.Relu, bias=B2[:, co:co + 1], scale=A2[:, co:co + 1])

                if stages < 9:
                    continue
                # ---- conv3x3 #2 + residual x += gamma*(h2 + ob2) ----
                wl2 = w2t[l % 2]
                for co in range(2):
                    nc.scalar.add(x[co][:], x[co][:], gob2t[:, co:co + 1])
                for half in range(2):
                    for co in range(2):
                        pss = [convps.tile([P, 512], f32, name="c2ps", tag="cps")
                               for _ in range(2)]
                        for tap in range(9):
                            ty, tx = tap // 3, tap % 3
                            for ci in range(2):
                                f = (tap * 2 + ci) * 2 + co
                                for ic in range(2):
                                    i0 = (half * 2 + ic) * RPC
                                    base = PW * (i0 + ty) + tx
                                    nc.tensor.matmul(
                                        pss[ic][:, 0:CW], wl2[:, f * P:(f + 1) * P],
                                        h1p[ci][:, base:base + CW],
                                        start=(tap == 0 and ci == 0),
                                        stop=(tap == 8 and ci == 1))
                        for ic in range(2):
                            i0 = (half * 2 + ic) * RPC
                            xslice = x[co][:, W * i0:W * (i0 + RPC)]
                            nc.vector.scalar_tensor_tensor(
                                out=xslice.rearrange("c (i j) -> c i j", j=W),
                                in0=pss[ic][:, 0:CW].rearrange(
                                    "c (i j) -> c i j", j=PW)[:, :, 0:W],
                                scalar=gamt[:],
                                in1=xslice.rearrange("c (i j) -> c i j", j=W),
                                op0=OP.mult, op1=OP.add)
                for co in range(2):
                    if l < layers - 1:
                        nc.scalar.copy(xb[co][:], x[co][:])
                    else:
                        if co == 0:
                            # consume the warmup-AllGather / warmup-matmul
                            # zeros (adds 0.0; keeps them from being DCE'd)
                            nc.vector.tensor_tensor(x[0][0:1, 0:8], x[0][0:1, 0:8],
                                                    wres[:], OP.add)
                            nc.vector.tensor_tensor(x[0][0:1, 0:8], x[0][0:1, 0:8],
                                                    wscr[0:1, 0:8], OP.add)
                        nc.sync.dma_start(out[co * P:(co + 1) * P, :], x[co][:])

    nc.compile()
    return nc


def _host_prep(inputs):
    bf = ml_dtypes.bfloat16
    kw, kb, qw, qb = inputs["kw"], inputs["kb"], inputs["qw"], inputs["qb"]
    vw, vb = inputs["vw"], inputs["vb"]
    ow1, ow2 = inputs["ow1"], inputs["ow2"]
    gammas, ob2 = inputs["gammas"], inputs["ob2"]

    def packw(wm):  # [L, KH, C] -> [L, 128, 2*KH]  (lhsT per ci-half, fused)
        a = wm.transpose(0, 2, 1).reshape(L, 2, P, KH)   # [L, ci, cin128, KH]
        return np.ascontiguousarray(a.transpose(0, 2, 1, 3).reshape(L, P, 2 * KH)
                                    ).astype(bf)

    d = {}
    d["wq"] = packw(qw / 8.0)
    d["wk"] = packw(kw)
    d["wv"] = packw(vw)
    d["bq"] = np.ascontiguousarray((qb / 8.0).reshape(L, 1, KH)).astype(bf)
    d["bk"] = np.ascontiguousarray(kb.reshape(L, 1, KH)).astype(bf)
    d["bv"] = np.ascontiguousarray(vb.reshape(L, 1, KH)).astype(bf)
    # ow1 [L, 256, 512, 3, 3] -> [L, cin128, f=(tap, ci4, co2), cout128]
    a1 = ow1.reshape(L, 2, P, 4, P, 3, 3)   # [L, co, cout, ci, cin, ty, tx]
    a1 = a1.transpose(0, 4, 5, 6, 3, 1, 2)  # [L, cin, ty, tx, ci, co, cout]
    d["w1d"] = np.ascontiguousarray(a1.reshape(L, P, 72 * P)).astype(bf)
    a2 = ow2.reshape(L, 2, P, 2, P, 3, 3)
    a2 = a2.transpose(0, 4, 5, 6, 3, 1, 2)  # [L, cin, ty, tx, ci, co, cout]
    d["w2d"] = np.ascontiguousarray(a2.reshape(L, P, 36 * P)).astype(bf)
    d["bngd"] = np.ascontiguousarray(
        inputs["bn_g"].reshape(L, 2, P).transpose(0, 2, 1)).astype(np.float32)
    d["bnbd"] = np.ascontiguousarray(
        inputs["bn_b"].reshape(L, 2, P).transpose(0, 2, 1)).astype(np.float32)
    gob2 = gammas[:, None] * ob2
    d["gob2d"] = np.ascontiguousarray(
        gob2.reshape(L, 2, P).transpose(0, 2, 1)).astype(np.float32)
    d["gamd"] = np.ascontiguousarray(
        np.broadcast_to(gammas[:, None, None], (L, P, 1))).astype(np.float32)
    d["identd"] = np.eye(P, dtype=np.float32).astype(bf)
    return d


def kernel(**inputs):
    if "nc" not in _compiled:
        _compiled["nc"] = _build()
    nc = _compiled["nc"]
    shared = _host_prep(inputs)
    x = np.ascontiguousarray(inputs["x"].reshape(B, C, HW)).astype(np.float32)
    in_maps = []
    for c in range(NC):
        m = dict(shared)
        m["xin"] = x[c]
        in_maps.append(m)
    res = bass_utils.run_bass_kernel_spmd(nc, in_maps, core_ids=list(range(NC)))
    outs = np.stack([res.results[c]["out"] for c in range(NC)])
    return outs.reshape(B, C, H, W).astype(np.float32)


# revision 57
# speedup vs baseline: 1.0979x; 1.0747x over previous
"""AttentiveDensenet Trainium2 Bass kernel (v2).

Data-parallel over batch B=8 across 8 NeuronCores (1 image per core).

v2 changes vs v1 (driven by trace analysis of the 1.52ms baseline):
  - Conv weights are host-packed so each layer's w1/w2 load as ONE big
    DMA each into resident SBUF tiles (double-buffered, prefetched a
    layer ahead). Kills the per-tile weight DMA storm (1392 SP DMA
    issues, 299k tiny packets) that starved the PE and kept the HAM
    clock gate cold (PE ran at 1.2 GHz for 87% of the run).
  - o is transposed to channel-major with PE transpose-mode matmuls
    (32x [128,128] blocks/layer) + ACT evac into the padded conv input,
    replacing the DRAM bounce + xbar-transpose chain (~40us/layer of
    dead time).
  - Attention is emitted per quarter-image (2 pos-blocks) and conv1 in
    8-row chunks interleaved with the o-transposes, so DVE attention
    overlaps PE conv work instead of serializing.
  - Weighted sum accumulates bf16 products with a pair tree (2x DVE
    mode) instead of sequential f32 adds.
  - A dummy AllGather at kernel start absorbs the ~47us core-launch
    skew barrier under layer-0 compute; per-layer BN stat AllGathers
    then run near their intrinsic latency.
  - BN stats are reduced per conv1-chunk as results land (DVE is idle
    then), so the AllGather starts immediately after the last chunk.
"""
import numpy as np
import ml_dtypes

import concourse.bacc as bacc
import concourse.mybir as mybir
import concourse.tile as tile
from concourse import bass_utils

L, C, B, H, W = 4, 256, 8, 32, 32
NH, KD = 8, 64
KH = NH * KD          # 512
HW = H * W            # 1024
P = 128
NC = 8                # cores
TOPK = 4
EPS = 1e-7
BN_EPS = 1e-5
PW = W + 2            # 34
PHW = PW * (H + 2)    # 1156
NQ = 4                # quarters (2 pos-blocks each)
RPC = 8               # conv chunk rows
NCH = H // RPC        # 4 conv chunks
CW = PW * RPC         # 272 conv chunk width (incl pad cols)

f32 = mybir.dt.float32
bf16 = mybir.dt.bfloat16
AX = mybir.AxisListType
OP = mybir.AluOpType
ACTF = mybir.ActivationFunctionType

_compiled = {}


def _build(ncores=NC, layers=L, stages=99):
    nc = bacc.Bacc(None, target_bir_lowering=False, debug=False, num_devices=ncores)

    # ---- DRAM I/O (per-core shapes; weights replicated) ----
    xin = nc.dram_tensor("xin", [C, HW], f32, kind="ExternalInput").ap()
    wq = nc.dram_tensor("wq", [L, P, 2 * KH], bf16, kind="ExternalInput").ap()
    wk = nc.dram_tensor("wk", [L, P, 2 * KH], bf16, kind="ExternalInput").ap()
    wv = nc.dram_tensor("wv", [L, P, 2 * KH], bf16, kind="ExternalInput").ap()
    bq = nc.dram_tensor("bq", [L, 1, KH], bf16, kind="ExternalInput").ap()
    bk = nc.dram_tensor("bk", [L, 1, KH], bf16, kind="ExternalInput").ap()
    bv = nc.dram_tensor("bv", [L, 1, KH], bf16, kind="ExternalInput").ap()
    w1d = nc.dram_tensor("w1d", [L, P, 72 * P], bf16, kind="ExternalInput").ap()
    w2d = nc.dram_tensor("w2d", [L, P, 36 * P], bf16, kind="ExternalInput").ap()
    bngd = nc.dram_tensor("bngd", [L, P, 2], f32, kind="ExternalInput").ap()
    bnbd = nc.dram_tensor("bnbd", [L, P, 2], f32, kind="ExternalInput").ap()
    gob2d = nc.dram_tensor("gob2d", [L, P, 2], f32, kind="ExternalInput").ap()
    gamd = nc.dram_tensor("gamd", [L, P, 1], f32, kind="ExternalInput").ap()
    identd = nc.dram_tensor("identd", [P, P], bf16, kind="ExternalInput").ap()
    out = nc.dram_tensor("out", [C, HW], f32, kind="ExternalOutput").ap()

    with tile.TileContext(nc) as tc:
        with tc.tile_pool(name="main", bufs=1) as mp, \
             tc.tile_pool(name="prodp", bufs=6) as prodp, \
             tc.tile_pool(name="sprod", bufs=2) as sprod, \
             tc.tile_pool(name="wkvp", bufs=4) as wkvp, \
             tc.tile_pool(name="biasp", bufs=3) as biasp, \
             tc.tile_pool(name="kqvps", bufs=3, space="PSUM") as kqvps, \
             tc.tile_pool(name="convps", bufs=3, space="PSUM") as convps, \
             tc.tile_pool(name="xps", bufs=2, space="PSUM") as xps, \
             tc.tile_pool(name="dramp", bufs=2, space="DRAM") as dramp:

            # ---- persistent tiles ----
            x = [mp.tile([P, HW], f32, name=f"x{i}") for i in range(2)]
            xb = [mp.tile([P, HW], bf16, name=f"xb{i}") for i in range(2)]
            qbt = mp.tile([P, 8 * KH], bf16, name="qbt")
            kbt = [mp.tile([P, 8 * KH], bf16, name=f"kbt{i}") for i in range(L)]
            vbt = [mp.tile([P, 8 * KH], bf16, name=f"vbt{i}") for i in range(L)]
            S = mp.tile([P, 64 * 5], f32, name="S")
            attn = mp.tile([P, 64 * 5], f32, name="attn")
            attnb = mp.tile([P, 64 * 5], bf16, name="attnb")
            mx = mp.tile([P, 64], f32, name="mx")
            mx2 = mp.tile([P, 64], f32, name="mx2")
            zs = mp.tile([P, 64], f32, name="zs")
            dmin = mp.tile([P, 64], f32, name="dmin")
            mxp = mp.tile([P, 64], f32, name="mxp")
            o = mp.tile([P, 8 * KH], bf16, name="o")
            opad = [mp.tile([P, PHW + 2], bf16, name=f"opad{i}") for i in range(4)]
            y1 = [mp.tile([P, HW], f32, name=f"y1_{i}") for i in range(2)]
            h1p = [mp.tile([P, PHW + 2], bf16, name=f"h1p{i}") for i in range(2)]
            st = mp.tile([P, 16], f32, name="st")       # (co, kind, chunk)
            st2 = mp.tile([P, 4], f32, name="st2")      # (co, kind)
            stT = mp.tile([4, P], f32, name="stT")      # transposed stats
            arT = mp.tile([4, P], f32, name="arT")      # all-reduced stats
            gsum = mp.tile([P, 4], f32, name="gsum")
            ones1 = mp.tile([1, P], bf16, name="ones1")
            ident = mp.tile([P, P], bf16, name="ident")
            identf = mp.tile([P, P], f32, name="identf")
            wres = mp.tile([1, 8], f32, name="wres")
            wsrc = mp.tile([1, 8], f32, name="wsrc")
            # resident conv weights, double-buffered across layers
            w1t = [mp.tile([P, 72 * P], bf16, name=f"w1t{i}") for i in range(2)]
            w2t = [mp.tile([P, 36 * P], bf16, name=f"w2t{i}") for i in range(2)]
            # per-layer consts (columns = co half)
            bngt = mp.tile([P, 2], f32, name="bngt")
            bnbt = mp.tile([P, 2], f32, name="bnbt")
            gob2t = mp.tile([P, 2], f32, name="gob2t")
            gamt = mp.tile([P, 1], f32, name="gamt")
            # BN scratch (columns = co half)
            mean2 = mp.tile([P, 2], f32, name="mean2")
            vart = mp.tile([P, 2], f32, name="vart")
            sq2 = mp.tile([P, 2], f32, name="sq2")
            stdt = mp.tile([P, 2], f32, name="stdt")
            A2 = mp.tile([P, 2], f32, name="A2")
            B2 = mp.tile([P, 2], f32, name="B2")

            # ---- init ----
            for i in range(2):
                nc.sync.dma_start(x[i][:], xin[i * P:(i + 1) * P, :])
                nc.scalar.copy(xb[i][:], x[i][:])
            nc.sync.dma_start(ident[:], identd)
            nc.vector.tensor_copy(identf[:], ident[:])
            for i in range(4):
                nc.vector.memset(opad[i][:], 0)
            for i in range(2):
                nc.vector.memset(h1p[i][:], 0)
            nc.vector.memset(ones1[:], 1.0)
            nc.vector.memset(S[:], 0)
            nc.vector.memset(attn[:], 0)
            nc.vector.memset(attnb[:], 0)
            nc.vector.memset(wsrc[:], 0)

            # PE warmup: ~5us of dummy matmuls (zeros) so the HAM clock
            # gate reaches 2.4 GHz before the first real KQV matmul.
            # Their (zero) result is consumed additively at the end.
            wscr = mp.tile([1, 512], bf16, name="wscr")
            for wi in range(10):
                wps = kqvps.tile([P, KH], f32, name="kqv_ps")
                nc.tensor.matmul(wps[:, 0:320], ones1[:], attnb[0:1, 0:320],
                                 start=True, stop=False)
                nc.tensor.matmul(wps[:, 0:320], ones1[:], attnb[0:1, 0:320],
                                 start=False, stop=True)
                if wi == 9:
                    nc.scalar.copy(wscr[0:1, 0:320], wps[0:1, 0:320])

            # warmup collective: absorbs core-launch skew + CC init
            # barrier while layer-0 compute runs. wres (all zeros) is
            # consumed additively just before the output DMA so DCE
            # can't drop the chain.
            wrmi = dramp.tile([1, 8], f32, name="wrmi")
            wrmo = dramp.tile([ncores, 8], f32, name="wrmo", addr_space="Shared")
            nc.sync.dma_start(wrmi[:], wsrc[:])
            nc.gpsimd.collective_compute(
                "AllGather", OP.bypass,
                replica_groups=[list(range(ncores))],
                ins=[wrmi.opt()], outs=[wrmo.opt()])
            nc.sync.dma_start(wres[:], wrmo[0:1, :])



            S3 = S[:].rearrange("p (g t) -> p g t", t=5)
            at3 = attn[:].rearrange("p (g t) -> p g t", t=5)
            ab3 = attnb[:].rearrange("p (g t) -> p g t", t=5)


            for l in range(layers):
                R = l + 1      # number of real keys
                T = R + 1      # +1 zero key

                # ---- per-layer consts ----
                nc.sync.dma_start(bngt[:], bngd[l])
                nc.sync.dma_start(bnbt[:], bnbd[l])
                nc.sync.dma_start(gob2t[:], gob2d[l])
                nc.sync.dma_start(gamt[:], gamd[l])

                # ---- K/Q/V 1x1 convs, position-major ----
                wts, bts = {}, {}
                for name, wdr, bdr in (("k", wk, bk), ("v", wv, bv), ("q", wq, bq)):
                    bt = biasp.tile([1, KH], bf16, name=f"bias_{name}", tag="bias")
                    nc.sync.dma_start(bt[:], bdr[l])
                    wt = wkvp.tile([P, 2 * KH], bf16, name=f"w_{name}", tag="wkv")
                    nc.sync.dma_start(wt[:], wdr[l])
                    wts[name], bts[name] = wt, bt
                if l == 0:
                    # layer-0 conv weights (after the KQV weight DMAs so
                    # those win the queue race; conv1 needs these ~40us in)
                    nc.sync.dma_start(w1t[0][:], w1d[0])
                    nc.sync.dma_start(w2t[0][:], w2d[0])
                dests = {"k": kbt[l][:], "v": vbt[l][:], "q": qbt[:]}
                for pb in range(8):
                    for name in ("k", "v", "q"):
                        ps = kqvps.tile([P, KH], f32, name="kqv_ps")
                        nc.tensor.matmul(ps[:], ones1[:], bts[name][:],
                                         start=True, stop=False)
                        nc.tensor.matmul(ps[:], xb[0][:, pb * P:(pb + 1) * P],
                                         wts[name][:, 0:KH], start=False, stop=False)
                        nc.tensor.matmul(ps[:], xb[1][:, pb * P:(pb + 1) * P],
                                         wts[name][:, KH:2 * KH], start=False, stop=True)
                        nc.scalar.copy(dests[name][:, pb * KH:(pb + 1) * KH], ps[:])

                if stages < 2:
                    continue

                # ---- attention, per quarter (2 pos-blocks) ----
                for qt in range(NQ):
                    g0 = 16 * qt
                    fs = 1024 * qt           # free-dim start in [128, 4096] tiles
                    qsl = slice(fs, fs + 1024)
                    # scores on DVE (gpsimd streaming measured 3-5x slower
                    # + port contention). The d-reduction folds twice in
                    # 2x-mode bf16 adds before the 1x tensor_reduce:
                    # 570+314+186+314 cyc vs 570+1140 for a flat reduce.
                    for t in range(R):
                        pr = sprod.tile([P, 1024], bf16, name="sprodt", tag="sp")
                        nc.vector.tensor_mul(pr[:], qbt[:, qsl], kbt[t][:, qsl])
                        sc = sprod.tile([P, 1024], bf16, name="sfold", tag="sp")
                        pr3 = pr[:].rearrange("p (g d) -> p g d", d=KD)
                        f1 = sc[:, 0:512].rearrange("p (g d) -> p g d", d=32)
                        nc.vector.tensor_tensor(f1, pr3[:, :, 0:32],
                                                pr3[:, :, 32:64], OP.add)
                        f2 = sc[:, 512:768].rearrange("p (g d) -> p g d", d=16)
                        nc.vector.tensor_tensor(f2, f1[:, :, 0:16],
                                                f1[:, :, 16:32], OP.add)
                        nc.vector.tensor_reduce(
                            out=S3[:, g0:g0 + 16, t], in_=f2,
                            axis=AX.X, op=OP.add)
                    nc.vector.memset(S3[:, g0:g0 + 16, R:R + 1], 0)  # zero key

                    if stages < 3:
                        continue
                    # softmax over T slots. No max-subtraction: scores are
                    # O(1) (weights are 0.02-scale, q pre-divided by temp),
                    # softmax is shift-invariant, f32 exp can't overflow.
                    zsq = zs[:, g0:g0 + 16]
                    nc.scalar.activation(at3[:, g0:g0 + 16, 0:T],
                                         S3[:, g0:g0 + 16, 0:T], ACTF.Exp)
                    if T <= TOPK:
                        nc.vector.tensor_reduce(out=zsq, in_=at3[:, g0:g0 + 16, 0:T],
                                                axis=AX.X, op=OP.add)
                        nc.vector.reciprocal(zsq, zsq)
                        nc.vector.tensor_tensor(
                            at3[:, g0:g0 + 16, 0:T], at3[:, g0:g0 + 16, 0:T],
                            zsq.unsqueeze(2).broadcast_to([P, 16, T]), OP.mult)
                    else:
                        # T=5 sparse top-k on the unnormalized exps (the
                        # topk renorm makes the first softmax normalization
                        # redundant up to an EPS*Z ~ 5e-7 shift).
                        # delta = 4th-largest = 2nd-smallest of 5, via a
                        # 10-op min/max network.
                        a = [at3[:, g0:g0 + 16, i] for i in range(T)]
                        u1 = dmin[:, g0:g0 + 16]
                        u2 = mxp[:, g0:g0 + 16]
                        u3 = mx[:, g0:g0 + 16]
                        u4 = mx2[:, g0:g0 + 16]
                        nc.vector.tensor_tensor(u1, a[0], a[1], OP.min)   # m1
                        nc.vector.tensor_tensor(u2, a[0], a[1], OP.max)   # M1
                        nc.vector.tensor_tensor(u3, a[2], a[3], OP.min)   # m2
                        nc.vector.tensor_tensor(u4, a[2], a[3], OP.max)   # M2
                        nc.vector.tensor_tensor(u2, u2, u4, OP.min)       # c
                        nc.vector.tensor_tensor(u4, u1, u3, OP.max)       # s3
                        nc.vector.tensor_tensor(u1, u1, u3, OP.min)       # m3
                        nc.vector.tensor_tensor(u2, u4, u2, OP.min)       # 2nd of 4
                        nc.vector.tensor_tensor(u3, u1, a[4], OP.max)
                        nc.vector.tensor_tensor(u1, u3, u2, OP.min)       # delta
                        nc.vector.tensor_scalar_add(u1, u1, EPS)
                        nc.vector.tensor_tensor(
                            at3[:, g0:g0 + 16, 0:T], at3[:, g0:g0 + 16, 0:T],
                            u1.unsqueeze(2).broadcast_to([P, 16, T]), OP.subtract)
                        nc.vector.tensor_scalar_max(at3[:, g0:g0 + 16, 0:T],
                                                    at3[:, g0:g0 + 16, 0:T], 0.0)
                        nc.vector.tensor_reduce(out=zsq, in_=at3[:, g0:g0 + 16, 0:T],
                                                axis=AX.X, op=OP.add)
                        nc.vector.tensor_scalar_add(zsq, zsq, EPS)
                        nc.vector.reciprocal(zsq, zsq)
                        nc.vector.tensor_tensor(
                            at3[:, g0:g0 + 16, 0:T], at3[:, g0:g0 + 16, 0:T],
                            zsq.unsqueeze(2).broadcast_to([P, 16, T]), OP.mult)

                    nc.scalar.copy(attnb[:, 80 * qt:80 * qt + 80],
                                   attn[:, 80 * qt:80 * qt + 80])

                    if stages < 4:
                        continue
                    # weighted sum: o_q = sum_t attn_t * v_t  (bf16 pair tree)
                    oq = o[:, qsl].rearrange("p (g d) -> p g d", d=KD)

                    def wprod(dst3, t):
                        nc.vector.tensor_tensor(
                            dst3, vbt[t][:, qsl].rearrange("p (g d) -> p g d", d=KD),
                            ab3[:, g0:g0 + 16, t].unsqueeze(2).broadcast_to(
                                [P, 16, KD]), OP.mult)

                    if R == 1:
                        wprod(oq, 0)
                    else:
                        pts = []
                        for t in range(R):
                            pt = prodp.tile([P, 1024], bf16, name="wprod", tag="wp")
                            wprod(pt[:].rearrange("p (g d) -> p g d", d=KD), t)
                            pts.append(pt)
                        if R == 2:
                            nc.vector.tensor_add(o[:, qsl], pts[0][:], pts[1][:])
                        elif R == 3:
                            t01 = prodp.tile([P, 1024], bf16, name="wprod", tag="wp")
                            nc.vector.tensor_add(t01[:], pts[0][:], pts[1][:])
                            nc.vector.tensor_add(o[:, qsl], t01[:], pts[2][:])
                        else:
                            t01 = prodp.tile([P, 1024], bf16, name="wprod", tag="wp")
                            t23 = prodp.tile([P, 1024], bf16, name="wprod", tag="wp")
                            nc.vector.tensor_add(t01[:], pts[0][:], pts[1][:])
                            nc.vector.tensor_add(t23[:], pts[2][:], pts[3][:])
                            nc.vector.tensor_add(o[:, qsl], t01[:], t23[:])

                    if stages < 5:
                        continue
                    # PE transpose o -> channel-major opad for this quarter
                    for pb in (2 * qt, 2 * qt + 1):
                        for ht in range(4):
                            tps = xps.tile([P, 1024], bf16, name="xpose_ps")
                            nc.tensor.transpose(
                                tps[:, 0:P],
                                o[:, pb * KH + ht * P:pb * KH + (ht + 1) * P],
                                ident[:])
                            opv = opad[ht][:, 0:PHW].rearrange(
                                "c (i j) -> c i j", j=PW)
                            nc.scalar.copy(
                                opv[:, 1 + 4 * pb:5 + 4 * pb, 1:W + 1],
                                tps[:, 0:P].rearrange("c (i j) -> c i j", j=W))

                if stages < 6:
                    continue
                # ---- conv3x3 #1: y1 = W1 * opad, 8-row chunks ----
                # weight-stationary over chunk pairs: both chunks of a
                # half run back-to-back per weight tile so LDWEIGHTS is
                # shared; half granularity keeps the attention overlap.
                wl1 = w1t[l % 2]
                for half in range(2):
                    for co in range(2):
                        pss = [convps.tile([P, 512], f32, name="c1ps", tag="cps")
                               for _ in range(2)]
                        for tap in range(9):
                            ty, tx = tap // 3, tap % 3
                            for ci in range(4):
                                f = (tap * 4 + ci) * 2 + co
                                for ic in range(2):
                                    i0 = (half * 2 + ic) * RPC
                                    base = PW * (i0 + ty) + tx
                                    nc.tensor.matmul(
                                        pss[ic][:, 0:CW], wl1[:, f * P:(f + 1) * P],
                                        opad[ci][:, base:base + CW],
                                        start=(tap == 0 and ci == 0),
                                        stop=(tap == 8 and ci == 3))
                        for ic in range(2):
                            c = half * 2 + ic
                            i0 = c * RPC
                            nc.scalar.copy(
                                y1[co][:, W * i0:W * (i0 + RPC)].rearrange(
                                    "c (i j) -> c i j", j=W),
                                pss[ic][:, 0:CW].rearrange(
                                    "c (i j) -> c i j", j=PW)[:, :, 0:W])
                            # per-chunk BN stats while DVE is otherwise idle
                            # st columns: (co*2 + kind)*4 + chunk
                            ysl = y1[co][:, W * i0:W * (i0 + RPC)]
                            i_sum = (co * 2 + 0) * 4 + c
                            i_sq = (co * 2 + 1) * 4 + c
                            nc.vector.tensor_reduce(out=st[:, i_sum:i_sum + 1],
                                                    in_=ysl, axis=AX.X, op=OP.add)
                            sqt = sprod.tile([P, 1024], bf16, name="sqt", tag="sp")
                            nc.scalar.square(sqt[:, 0:W * RPC], ysl)
                            nc.vector.tensor_reduce(out=st[:, i_sq:i_sq + 1],
                                                    in_=sqt[:, 0:W * RPC],
                                                    axis=AX.X, op=OP.add)

                if stages < 7:
                    continue
                # ---- BN stats AllGather ----
                # PE-transpose the [128,4] stats to [4,128] so both DMA
                # legs move contiguous 512B rows (the naive layouts cost
                # ~8us each in 16B/4B scattered packets).
                nc.vector.tensor_reduce(
                    out=st2[:], in_=st[:].rearrange("p (g c) -> p g c", c=4),
                    axis=AX.X, op=OP.add)
                tp0 = kqvps.tile([4, P], f32, name="stT_ps", tag="kqv_ps")
                nc.tensor.transpose(tp0[:], st2[:], identf[:])
                nc.scalar.copy(stT[:], tp0[:])
                cci = dramp.tile([4, P], f32, name="cci")
                cco = dramp.tile([4, P], f32, name="cco", addr_space="Shared")
                nc.sync.dma_start(cci[:], stT[:])
                nc.gpsimd.collective_compute(
                    "AllReduce", OP.add,
                    replica_groups=[list(range(ncores))],
                    ins=[cci.opt()], outs=[cco.opt()])
                nc.sync.dma_start(arT[:], cco[:])
                # transpose the summed [4,128] stats back to [128,4]
                tpb = kqvps.tile([P, 32], f32, name="gsum_ps", tag="kqv_ps")
                nc.tensor.transpose(tpb[:, 0:4], arT[:], identf[0:4, 0:4])
                nc.vector.tensor_copy(gsum[:], tpb[:, 0:4])

                # prefetch next layer's conv weights (overlaps conv2 + next KQV)
                if l + 1 < layers:
                    nc.sync.dma_start(w1t[(l + 1) % 2][:], w1d[l + 1])
                    nc.sync.dma_start(w2t[(l + 1) % 2][:], w2d[l + 1])

                if stages < 8:
                    continue
                # ---- BN coefficients: A = g/sqrt(var+eps), B = b - mean*A ----
                # computed for both co halves at once on [P,2] tiles
                NTOT = float(ncores * HW)
                g4 = gsum[:].rearrange("p (co k) -> p co k", k=2)
                nc.vector.tensor_scalar_mul(mean2[:], g4[:, :, 0], 1.0 / NTOT)
                nc.vector.tensor_scalar_mul(vart[:], g4[:, :, 1], 1.0 / NTOT)
                nc.vector.tensor_mul(sq2[:], mean2[:], mean2[:])
                nc.vector.tensor_sub(vart[:], vart[:], sq2[:])
                nc.vector.tensor_scalar_add(vart[:], vart[:], BN_EPS)
                nc.scalar.activation(stdt[:], vart[:], ACTF.Sqrt)
                nc.vector.reciprocal(stdt[:], stdt[:])
                nc.vector.tensor_mul(A2[:], bngt[:], stdt[:])
                nc.vector.tensor_mul(sq2[:], mean2[:], A2[:])
                nc.vector.tensor_sub(B2[:], bnbt[:], sq2[:])
                # h1 = relu(A*y1 + B), strided bf16 into padded conv2 input;
                # split in row halves so conv2's first chunks start sooner
                for rh in range(2):
                    r0 = rh * (H // 2)
                    for co in range(2):
                        h1v = h1p[co][:, 0:PHW].rearrange("c (i j) -> c i j", j=PW)
                        y1v = y1[co][:].rearrange("c (i j) -> c i j", j=W)
                        nc.scalar.activation(
                            h1v[:, 1 + r0:1 + r0 + H // 2, 1:W + 1],
                            y1v[:, r0:r0 + H // 2, :],
                            ACTF.Relu, bias=B2[:, co:co + 1], scale=A2[:, co:co + 1])

                if stages < 9:
                    continue
                # ---- conv3x3 #2 + residual x += gamma*(h2 + ob2) ----
                wl2 = w2t[l % 2]
                for co in range(2):
                    nc.scalar.add(x[co][:], x[co][:], gob2t[:, co:co + 1])
                for half in range(2):
                    for co in range(2):
                        pss = [convps.tile([P, 512], f32, name="c2ps", tag="cps")
                               for _ in range(2)]
                        for tap in range(9):
                            ty, tx = tap // 3, tap % 3
                            for ci in range(2):
                                f = (tap * 2 + ci) * 2 + co
                                for ic in range(2):
                                    i0 = (half * 2 + ic) * RPC
                                    base = PW * (i0 + ty) + tx
                                    nc.tensor.matmul(
                                        pss[ic][:, 0:CW], wl2[:, f * P:(f + 1) * P],
                                        h1p[ci][:, base:base + CW],
                                        start=(tap == 0 and ci == 0),
                                        stop=(tap == 8 and ci == 1))
                        for ic in range(2):
                            i0 = (half * 2 + ic) * RPC
                            xslice = x[co][:, W * i0:W * (i0 + RPC)]
                            nc.vector.scalar_tensor_tensor(
                                out=xslice.rearrange("c (i j) -> c i j", j=W),
                                in0=pss[ic][:, 0:CW].rearrange(
                                    "c (i j) -> c i j", j=PW)[:, :, 0:W],
                                scalar=gamt[:],
                                in1=xslice.rearrange("c (i j) -> c i j", j=W),
                                op0=OP.mult, op1=OP.add)
                for co in range(2):
                    if l < layers - 1:
                        nc.scalar.copy(xb[co][:], x[co][:])
                    else:
                        if co == 0:
                            # consume the warmup-AllGather / warmup-matmul
                            # zeros (adds 0.0; keeps them from being DCE'd)
                            nc.vector.tensor_tensor(x[0][0:1, 0:8], x[0][0:1, 0:8],
                                                    wres[:], OP.add)
                            nc.vector.tensor_tensor(x[0][0:1, 0:8], x[0][0:1, 0:8],
                                                    wscr[0:1, 0:8], OP.add)
                        nc.sync.dma_start(out[co * P:(co + 1) * P, :], x[co][:])

    nc.compile()
    return nc


def _host_prep(inputs):
    bf = ml_dtypes.bfloat16
    kw, kb, qw, qb = inputs["kw"], inputs["kb"], inputs["qw"], inputs["qb"]
    vw, vb = inputs["vw"], inputs["vb"]
    ow1, ow2 = inputs["ow1"], inputs["ow2"]
    gammas, ob2 = inputs["gammas"], inputs["ob2"]

    def packw(wm):  # [L, KH, C] -> [L, 128, 2*KH]  (lhsT per ci-half, fused)
        a = wm.transpose(0, 2, 1).reshape(L, 2, P, KH)   # [L, ci, cin128, KH]
        return np.ascontiguousarray(a.transpose(0, 2, 1, 3).reshape(L, P, 2 * KH)
                                    ).astype(bf)

    d = {}
    d["wq"] = packw(qw / 8.0)
    d["wk"] = packw(kw)
    d["wv"] = packw(vw)
    d["bq"] = np.ascontiguousarray((qb / 8.0).reshape(L, 1, KH)).astype(bf)
    d["bk"] = np.ascontiguousarray(kb.reshape(L, 1, KH)).astype(bf)
    d["bv"] = np.ascontiguousarray(vb.reshape(L, 1, KH)).astype(bf)
    # ow1 [L, 256, 512, 3, 3] -> [L, cin128, f=(tap, ci4, co2), cout128]
    a1 = ow1.reshape(L, 2, P, 4, P, 3, 3)   # [L, co, cout, ci, cin, ty, tx]
    a1 = a1.transpose(0, 4, 5, 6, 3, 1, 2)  # [L, cin, ty, tx, ci, co, cout]
    d["w1d"] = np.ascontiguousarray(a1.reshape(L, P, 72 * P)).astype(bf)
    a2 = ow2.reshape(L, 2, P, 2, P, 3, 3)
    a2 = a2.transpose(0, 4, 5, 6, 3, 1, 2)  # [L, cin, ty, tx, ci, co, cout]
    d["w2d"] = np.ascontiguousarray(a2.reshape(L, P, 36 * P)).astype(bf)
    d["bngd"] = np.ascontiguousarray(
        inputs["bn_g"].reshape(L, 2, P).transpose(0, 2, 1)).astype(np.float32)
    d["bnbd"] = np.ascontiguousarray(
        inputs["bn_b"].reshape(L, 2, P).transpose(0, 2, 1)).astype(np.float32)
    gob2 = gammas[:, None] * ob2
    d["gob2d"] = np.ascontiguousarray(
        gob2.reshape(L, 2, P).transpose(0, 2, 1)).astype(np.float32)
    d["gamd"] = np.ascontiguousarray(
        np.broadcast_to(gammas[:, None, None], (L, P, 1))).astype(np.float32)
    d["identd"] = np.eye(P, dtype=np.float32).astype(bf)
    return d


def kernel(**inputs):
    if "nc" not in _compiled:
        _compiled["nc"] = _build()
    nc = _compiled["nc"]
    shared = _host_prep(inputs)
    x = np.ascontiguousarray(inputs["x"].reshape(B, C, HW)).astype(np.float32)
    in_maps = []
    for c in range(NC):
        m = dict(shared)
        m["xin"] = x[c]
        in_maps.append(m)
    res = bass_utils.run_bass_kernel_spmd(nc, in_maps, core_ids=list(range(NC)))
    outs = np.stack([res.results[c]["out"] for c in range(NC)])
    return outs.reshape(B, C, H, W).astype(np.float32)
